# revision 2
# baseline (speedup 1.0000x reference)
"""Trainium2 Bass kernel: ConvNeXt MLP + parallel MoE-LoRA (data-parallel over tokens).

Math per token t (D=512, Dh=2048, E=3 experts, r=8, top-k=2):
    base = gelu(x @ W1 + b1) @ W2 + b2
    g_e  = gelu(x @ w_down[e]) * wts[e, t]          (wts from top-k routing)
    out  = base + sum_e g_e @ w_up[e]

v2 strategy (per NeuronCore, 8 cores data-parallel on the token dim):
  Fully feature-major ("transposed") dataflow so the PE does nothing but
  512-free-dim bf16 matmuls back to back:
    - host ships x^T [D, Tc] pre-cast to bf16 -> no on-chip transposes/casts.
    - MM1: hT[h, t] = W1_chunk.T @ xT, ScalarE fuses +b1 and gelu, writing
      actT in bf16 (feature-major).
    - MM2 computes the TRANSPOSED output: outT[d, t] = W2_chunk.T @ actT
      accumulated over 16 h-chunks, plus the rank-24 LoRA 'up' matmul in the
      same PSUM group.  b2 becomes a per-partition scalar -> fused into the
      DVE drain (PSUM -> bf16 SBUF -> DMA out as outT [D, Tc]).
    - LoRA: gT[24, t] = gelu(wdown_all.T @ xT) on 24 partitions, scaled by
      routing weights with one DVE tensor_tensor against a host-prepared
      wtsE [24, Tc] tile.
    - routing weights (mask-combine of topk_probs/indices) and all weight
      layout/casts are host-side prep; output is transposed back on host.
  PE work per 512-token group: 4 (lora down) + 64 (MM1) + 4x17 (MM2) = 136
  bf16 matmuls, free dim 512, no other PE instructions.
"""

import os
import numpy as np

P = 128
D = 512
DH = 2048
E = 3
R = 8
ER = E * R  # 24
NH = DH // P  # 16
NDC = D // P  # 4
N_CORES = 8
T_FULL = 64 * 28 * 28  # 50176
TC = T_FULL // N_CORES  # 6272
GROUP_TILES = 4
G_MAX = GROUP_TILES * P  # 512

_CACHE = {}


def _build(tc_tokens):
    import concourse.bacc as bacc
    import concourse.tile as tile
    import concourse.mybir as mybir
    from contextlib import ExitStack

    f32 = mybir.dt.float32
    bf16 = mybir.dt.bfloat16
    AF = mybir.ActivationFunctionType
    OP = mybir.AluOpType

    nt = tc_tokens // P
    assert tc_tokens % P == 0

    nc = bacc.Bacc("TRN2", target_bir_lowering=False, debug=False,
                   num_devices=N_CORES)

    xT = nc.dram_tensor("xT", [D, tc_tokens], bf16, kind="ExternalInput").ap()
    w1 = nc.dram_tensor("w1", [D, DH], bf16, kind="ExternalInput").ap()
    w2 = nc.dram_tensor("w2", [DH, D], bf16, kind="ExternalInput").ap()
    b1 = nc.dram_tensor("b1", [P, NH], f32, kind="ExternalInput").ap()
    b2 = nc.dram_tensor("b2", [P, NDC], f32, kind="ExternalInput").ap()
    wd = nc.dram_tensor("wd", [D, ER], bf16, kind="ExternalInput").ap()
    wu = nc.dram_tensor("wu", [ER, D], bf16, kind="ExternalInput").ap()
    wtse = nc.dram_tensor("wtse", [ER, tc_tokens], f32, kind="ExternalInput").ap()
    outT = nc.dram_tensor("outT", [D, tc_tokens], bf16, kind="ExternalOutput").ap()

    # group schedule: full 512-token groups + one remainder group
    groups = []
    t0 = 0
    while t0 < nt:
        ng = min(GROUP_TILES, nt - t0)
        groups.append((t0 * P, ng * P))
        t0 += ng

    xT_r = xT.rearrange("(c p) t -> p c t", p=P)
    outT_r = outT.rearrange("(c p) t -> p c t", p=P)

    with tile.TileContext(nc) as tc, ExitStack() as ctx:
        cons = ctx.enter_context(tc.tile_pool(name="cons", bufs=1))
        xin = ctx.enter_context(tc.tile_pool(name="xin", bufs=4))
        actp = ctx.enter_context(tc.tile_pool(name="actp", bufs=2))
        gp = ctx.enter_context(tc.tile_pool(name="gp", bufs=3))
        outp = ctx.enter_context(tc.tile_pool(name="outp", bufs=6))
        ps_h = ctx.enter_context(tc.tile_pool(name="ps_h", bufs=2, space="PSUM"))
        ps_o = ctx.enter_context(tc.tile_pool(name="ps_o", bufs=2, space="PSUM"))
        ps_g = ctx.enter_context(tc.tile_pool(name="ps_g", bufs=2, space="PSUM"))

        # ---- weight / const DMAs (w1 first: first MM1 group needs it) ----
        w1_sb = cons.tile([P, NDC * DH], bf16)
        w1_rv = w1.rearrange("(c p) h -> p c h", p=P)
        for c in range(NDC):
            nc.sync.dma_start(w1_sb[:, c * DH:(c + 1) * DH], w1_rv[:, c, :])

        # x group 0 + 1 prefetch (split per chunk for queue parallelism)
        x_tiles = {}

        def fetch_group(gi):
            gt0, gG = groups[gi]
            xt = xin.tile([P, NDC * gG], bf16, tag="xT", name=f"xT{gi}")
            for c in range(NDC):
                nc.sync.dma_start(xt[:, c * gG:(c + 1) * gG],
                                  xT_r[:, c, gt0:gt0 + gG])
            x_tiles[gi] = xt

        fetch_group(0)

        wd_sb = cons.tile([P, NDC * ER], bf16)
        nc.sync.dma_start(wd_sb.rearrange("p (c r) -> p c r", c=NDC),
                          wd.rearrange("(c p) r -> p c r", p=P))
        b1_sb = cons.tile([P, NH], f32)
        nc.sync.dma_start(b1_sb[:], b1)
        b2_sb = cons.tile([P, NDC], f32)
        nc.sync.dma_start(b2_sb[:], b2)
        wu_sb = cons.tile([ER, D], bf16)
        nc.sync.dma_start(wu_sb[:], wu)

        fetch_group(1)

        w2_sb = cons.tile([P, NH * D], bf16)
        w2_rv = w2.rearrange("(hc p) d -> p hc d", p=P)
        for h4 in range(4):
            nc.sync.dma_start(
                w2_sb.rearrange("p (hc d) -> p hc d", hc=NH)[:, h4 * 4:(h4 + 1) * 4, :],
                w2_rv[:, h4 * 4:(h4 + 1) * 4, :])

        wtse_sb = cons.tile([ER, tc_tokens], f32)
        nc.sync.dma_start(wtse_sb[:], wtse)

        if len(groups) > 2:
            fetch_group(2)

        # ---- PE warm-up: tiny matmuls on a memset tile during DMA wait ----
        warm = cons.tile([P, 16], bf16)
        nc.vector.memset(warm[:], 0.0)
        ps_w = ps_g.tile([16, 16], f32, tag="ps_g", name="warm")
        for i in range(40):
            nc.tensor.matmul(ps_w[:], warm[:, :16], warm[:, :16],
                             start=True, stop=True)

        # ---- main loop over groups ----
        for gi, (gt0, G) in enumerate(groups):
            if gi + 3 < len(groups) and (gi + 3) not in x_tiles:
                fetch_group(gi + 3)
            xt = x_tiles.pop(gi)

            # LoRA down first: its ScalarE/DVE chain runs under MM1
            pg = ps_g.tile([ER, G_MAX], f32, tag="ps_g", name=f"pg{gi}")
            for c in range(NDC):
                nc.tensor.matmul(pg[:, :G],
                                 wd_sb[:, c * ER:(c + 1) * ER],
                                 xt[:, c * G:(c + 1) * G],
                                 start=(c == 0), stop=(c == NDC - 1))
            g_f = gp.tile([ER, G_MAX], f32, tag="g_f", name=f"g_f{gi}")
            nc.scalar.activation(g_f[:, :G], pg[:, :G], AF.Gelu)
            g2 = gp.tile([ER, G_MAX], bf16, tag="g2", name=f"g2{gi}")
            nc.vector.tensor_tensor(g2[:, :G], g_f[:, :G],
                                    wtse_sb[:, gt0:gt0 + G], op=OP.mult)

            # MM1 + fused bias+gelu -> actT bf16 (feature-major)
            actT = actp.tile([P, NH * G], bf16, tag="actT", name=f"actT{gi}")
            for h in range(NH):
                ph = ps_h.tile([P, G_MAX], f32, tag="ps_h", name=f"ph{gi}_{h}")
                for c in range(NDC):
                    nc.tensor.matmul(
                        ph[:, :G],
                        w1_sb[:, c * DH + h * P: c * DH + (h + 1) * P],
                        xt[:, c * G:(c + 1) * G],
                        start=(c == 0), stop=(c == NDC - 1))
                nc.scalar.activation(actT[:, h * G:(h + 1) * G], ph[:, :G],
                                     AF.Gelu, bias=b1_sb[:, h:h + 1], scale=1.0)

            # MM2: outT[d, t] over 4 d-chunks; LoRA-up opens each PSUM group
            for dc in range(NDC):
                po = ps_o.tile([P, G_MAX], f32, tag="ps_o", name=f"po{gi}_{dc}")
                nc.tensor.matmul(po[:, :G],
                                 wu_sb[:, dc * P:(dc + 1) * P],
                                 g2[:, :G],
                                 start=True, stop=False)
                for h in range(NH):
                    nc.tensor.matmul(
                        po[:, :G],
                        w2_sb[:, h * D + dc * P: h * D + (dc + 1) * P],
                        actT[:, h * G:(h + 1) * G],
                        start=False, stop=(h == NH - 1))
                o_sb = outp.tile([P, G_MAX], bf16, tag="o_sb", name=f"o{gi}_{dc}")
                nc.vector.tensor_scalar(o_sb[:, :G], po[:, :G],
                                        b2_sb[:, dc:dc + 1], None, op0=OP.add)
                nc.sync.dma_start(outT_r[:, dc, gt0:gt0 + G], o_sb[:, :G])

    nc.compile()
    return nc


def _get_nc():
    key = ("v2", TC)
    if key not in _CACHE:
        _CACHE[key] = _build(TC)
    return _CACHE[key]


def _make_in_maps(inputs, tc_tokens=TC, n_cores=N_CORES):
    import ml_dtypes
    bf = ml_dtypes.bfloat16

    x = np.ascontiguousarray(inputs["x"], dtype=np.float32)
    T = x.size // D
    xT_full = np.ascontiguousarray(x.reshape(T, D).T.astype(bf))  # [D, T]
    W1 = np.ascontiguousarray(np.asarray(inputs["W1"], dtype=np.float32).astype(bf))
    W2 = np.ascontiguousarray(np.asarray(inputs["W2"], dtype=np.float32).astype(bf))
    b1 = np.ascontiguousarray(
        np.asarray(inputs["b1"], dtype=np.float32).reshape(NH, P).T)  # [P, NH]
    b2 = np.ascontiguousarray(
        np.asarray(inputs["b2"], dtype=np.float32).reshape(NDC, P).T)  # [P, NDC]
    wdn = np.ascontiguousarray(
        np.asarray(inputs["w_down"], dtype=np.float32)
        .transpose(1, 0, 2).reshape(D, ER).astype(bf))
    wup = np.ascontiguousarray(
        np.asarray(inputs["w_up"], dtype=np.float32).reshape(ER, D).astype(bf))

    # routing weights: wts[e, t] = sum_k topk_probs[t, k] * (topk_indices[t, k] == e)
    tkp = np.asarray(inputs["topk_probs"], dtype=np.float32)  # [T, K]
    tki = np.asarray(inputs["topk_indices"])  # [T, K]
    wts = np.zeros((E, T), dtype=np.float32)
    for e in range(E):
        wts[e] = np.sum(np.where(tki == e, tkp, 0.0), axis=1)
    wtse_full = np.repeat(wts, R, axis=0)  # [ER, T]

    in_maps = []
    for c in range(n_cores):
        sl = slice(c * tc_tokens, (c + 1) * tc_tokens)
        in_maps.append(dict(
            xT=np.ascontiguousarray(xT_full[:, sl]),
            w1=W1, w2=W2, b1=b1, b2=b2, wd=wdn, wu=wup,
            wtse=np.ascontiguousarray(wtse_full[:, sl])))
    return in_maps


def _ensure_ntff_hook():
    """Register the axon NTFF profile hook if the image's antenv lacks it."""
    import sys
    import types
    try:
        from antenv.axon_hooks import get_axon_ntff_profile_hook  # noqa: F401
        return True
    except ImportError:
        pass
    try:
        from trn_agent_boot.trn_boot import _ntff_profile_via_ctypes
        mod = types.ModuleType("antenv.axon_hooks")
        _hook = [None]
        mod.set_axon_ntff_profile_hook = lambda h: _hook.__setitem__(0, h)
        mod.get_axon_ntff_profile_hook = lambda: _hook[0]
        sys.modules["antenv.axon_hooks"] = mod
        import antenv
        antenv.axon_hooks = mod
        mod.set_axon_ntff_profile_hook(
            _ntff_profile_via_ctypes("/opt/axon/libaxon_pjrt.so"))
        return True
    except Exception:
        return False


def kernel(**inputs):
    from concourse.bass_utils import run_bass_kernel_spmd

    nc = _get_nc()
    in_maps = _make_in_maps(inputs)
    trace = bool(int(os.environ.get("KERNEL_TRACE", "0")))
    if trace and not _ensure_ntff_hook():
        trace = False
    res = run_bass_kernel_spmd(nc, in_maps, list(range(N_CORES)), trace=trace)
    if trace:
        _CACHE["last_result"] = res
    outT = np.concatenate(
        [np.asarray(res.results[i]["outT"]) for i in range(N_CORES)], axis=1)
    out = outT.astype(np.float32).T  # [T, D]
    return np.ascontiguousarray(out).reshape(np.asarray(inputs["x"]).shape)


# revision 14
# speedup vs baseline: 1.0466x; 1.0466x over previous
"""Trainium2 Bass kernel: ConvNeXt MLP + parallel MoE-LoRA (data-parallel over tokens).

Math per token t (D=512, Dh=2048, E=3 experts, r=8, top-k=2):
    base = gelu(x @ W1 + b1) @ W2 + b2
    g_e  = gelu(x @ w_down[e]) * wts[e, t]          (wts from top-k routing)
    out  = base + sum_e g_e @ w_up[e]

v2 strategy (per NeuronCore, 8 cores data-parallel on the token dim):
  Fully feature-major ("transposed") dataflow so the PE does nothing but
  512-free-dim bf16 matmuls back to back:
    - host ships x^T [D, Tc] pre-cast to bf16 -> no on-chip transposes/casts.
    - MM1: hT[h, t] = W1_chunk.T @ xT, ScalarE fuses +b1 and gelu, writing
      actT in bf16 (feature-major).
    - MM2 computes the TRANSPOSED output: outT[d, t] = W2_chunk.T @ actT
      accumulated over 16 h-chunks, plus the rank-24 LoRA 'up' matmul in the
      same PSUM group.  b2 becomes a per-partition scalar -> fused into the
      DVE drain (PSUM -> bf16 SBUF -> DMA out as outT [D, Tc]).
    - LoRA: gT[24, t] = gelu(wdown_all.T @ xT) on 24 partitions, scaled by
      routing weights with one DVE tensor_tensor against a host-prepared
      wtsE [24, Tc] tile.
    - routing weights (mask-combine of topk_probs/indices) and all weight
      layout/casts are host-side prep; output is transposed back on host.
  PE work per 512-token group: 4 (lora down) + 64 (MM1) + 4x17 (MM2) = 136
  bf16 matmuls, free dim 512, no other PE instructions.
"""

import os
import numpy as np

P = 128
D = 512
DH = 2048
E = 3
R = 8
ER = E * R  # 24
NH = DH // P  # 16
NDC = D // P  # 4
N_CORES = 8
T_FULL = 64 * 28 * 28  # 50176
TC = T_FULL // N_CORES  # 6272
GROUP_TILES = 4
G_MAX = GROUP_TILES * P  # 512

_CACHE = {}


def _build(tc_tokens):
    import concourse.bacc as bacc
    import concourse.tile as tile
    import concourse.mybir as mybir
    from contextlib import ExitStack

    f32 = mybir.dt.float32
    bf16 = mybir.dt.bfloat16
    AF = mybir.ActivationFunctionType
    OP = mybir.AluOpType

    nt = tc_tokens // P
    assert tc_tokens % P == 0

    nc = bacc.Bacc("TRN2", target_bir_lowering=False, debug=False,
                   num_devices=N_CORES)

    xT = nc.dram_tensor("xT", [D, tc_tokens], bf16, kind="ExternalInput").ap()
    w1 = nc.dram_tensor("w1", [D, DH], bf16, kind="ExternalInput").ap()
    w2 = nc.dram_tensor("w2", [DH, D], bf16, kind="ExternalInput").ap()
    b1 = nc.dram_tensor("b1", [P, NH], f32, kind="ExternalInput").ap()
    b2 = nc.dram_tensor("b2", [P, NDC], f32, kind="ExternalInput").ap()
    # LoRA weights zero-padded to full 128 partitions/columns on host so the
    # LoRA matmuls are uniform 128x128x512 (avoids LDWEIGHTS row/col-group
    # conflict stalls against the neighbouring full-size matmuls)
    wd = nc.dram_tensor("wd", [D, P], bf16, kind="ExternalInput").ap()
    wu = nc.dram_tensor("wu", [P, D], bf16, kind="ExternalInput").ap()
    wtse = nc.dram_tensor("wtse", [32, tc_tokens], f32, kind="ExternalInput").ap()
    outT = nc.dram_tensor("outT", [D, tc_tokens], bf16, kind="ExternalOutput").ap()

    # group schedule: full 512-token groups + one remainder group
    groups = []
    t0 = 0
    while t0 < nt:
        ng = min(GROUP_TILES, nt - t0)
        groups.append((t0 * P, ng * P))
        t0 += ng

    xT_r = xT.rearrange("(c p) t -> p c t", p=P)
    outT_r = outT.rearrange("(c p) t -> p c t", p=P)

    with tile.TileContext(nc) as tc, ExitStack() as ctx:
        cons = ctx.enter_context(tc.tile_pool(name="cons", bufs=1))
        xin = ctx.enter_context(tc.tile_pool(name="xin", bufs=4))
        actp = ctx.enter_context(tc.tile_pool(name="actp", bufs=2))
        gp = ctx.enter_context(tc.tile_pool(name="gp", bufs=3))
        outp = ctx.enter_context(tc.tile_pool(name="outp", bufs=6))
        ps_h = ctx.enter_context(tc.tile_pool(name="ps_h", bufs=4, space="PSUM"))
        ps_o = ctx.enter_context(tc.tile_pool(name="ps_o", bufs=2, space="PSUM"))
        ps_g = ctx.enter_context(tc.tile_pool(name="ps_g", bufs=1, space="PSUM"))
        ps_w = ctx.enter_context(tc.tile_pool(name="ps_w", bufs=1, space="PSUM"))

        # ---- weight / const DMAs (w1 first: first MM1 group needs it) ----
        # w1 split into (h-quarter, c) pieces, h-quarter-major, so MM1 of
        # group 0 can start as soon as the first quarter lands
        w1_sb = cons.tile([P, NDC * DH], bf16)
        w1_rv = w1.rearrange("(c p) h -> p c h", p=P)
        HQ = DH // 4
        for hq in range(4):
            for c in range(NDC):
                nc.sync.dma_start(
                    w1_sb[:, c * DH + hq * HQ: c * DH + (hq + 1) * HQ],
                    w1_rv[:, c, hq * HQ:(hq + 1) * HQ])

        # x group 0 + 1 prefetch (split per chunk for queue parallelism)
        x_tiles = {}

        def fetch_group(gi):
            gt0, gG = groups[gi]
            xt = xin.tile([P, NDC * gG], bf16, tag="xT", name=f"xT{gi}")
            for c in range(NDC):
                nc.sync.dma_start(xt[:, c * gG:(c + 1) * gG],
                                  xT_r[:, c, gt0:gt0 + gG])
            x_tiles[gi] = xt

        fetch_group(0)

        wd_sb = cons.tile([P, NDC * P], bf16)
        nc.sync.dma_start(wd_sb.rearrange("p (c r) -> p c r", c=NDC),
                          wd.rearrange("(c p) r -> p c r", p=P))
        b1_sb = cons.tile([P, NH], f32)
        nc.sync.dma_start(b1_sb[:], b1)
        b2_sb = cons.tile([P, NDC], f32)
        nc.sync.dma_start(b2_sb[:], b2)
        wu_sb = cons.tile([P, D], bf16)
        nc.sync.dma_start(wu_sb[:], wu)

        fetch_group(1)

        w2_sb = cons.tile([P, NH * D], bf16)
        w2_rv = w2.rearrange("(hc p) d -> p hc d", p=P)
        for h4 in range(4):
            nc.sync.dma_start(
                w2_sb.rearrange("p (hc d) -> p hc d", hc=NH)[:, h4 * 4:(h4 + 1) * 4, :],
                w2_rv[:, h4 * 4:(h4 + 1) * 4, :])

        wtse_sb = cons.tile([32, tc_tokens], f32)
        nc.sync.dma_start(wtse_sb[:], wtse)

        if len(groups) > 2:
            fetch_group(2)

        # ---- PE warm-up: tiny matmuls on a memset tile during DMA wait ----
        warm = cons.tile([P, 16], bf16)
        nc.vector.memset(warm[:], 0.0)
        psw = ps_w.tile([16, 16], f32, tag="warm", name="warm")
        for i in range(40):
            nc.tensor.matmul(psw[:], warm[:, :16], warm[:, :16],
                             start=True, stop=True)

        # ---- main loop over groups ----
        for gi, (gt0, G) in enumerate(groups):
            if gi + 3 < len(groups) and (gi + 3) not in x_tiles:
                fetch_group(gi + 3)
            xt = x_tiles.pop(gi)

            # LoRA down first: its ScalarE/DVE chain runs under MM1
            pg = ps_g.tile([P, G_MAX], f32, tag="ps_g", name=f"pg{gi}")
            for c in range(NDC):
                nc.tensor.matmul(pg[:, :G],
                                 wd_sb[:, c * P:(c + 1) * P],
                                 xt[:, c * G:(c + 1) * G],
                                 start=(c == 0), stop=(c == NDC - 1))
            g_f = gp.tile([32, G_MAX], f32, tag="g_f", name=f"g_f{gi}")
            nc.scalar.activation(g_f[:, :G], pg[:32, :G], AF.Gelu)
            g2 = gp.tile([P, G_MAX], bf16, tag="g2", name=f"g2{gi}")
            nc.vector.memset(g2[:, :G], 0.0)
            nc.vector.tensor_tensor(g2[:32, :G], g_f[:, :G],
                                    wtse_sb[:, gt0:gt0 + G], op=OP.mult)

            # MM1 + fused bias+gelu -> actT bf16 (feature-major)
            actT = actp.tile([P, NH * G], bf16, tag="actT", name=f"actT{gi}")
            for h in range(NH):
                ph = ps_h.tile([P, G_MAX], f32, tag="ps_h", name=f"ph{gi}_{h}")
                for c in range(NDC):
                    nc.tensor.matmul(
                        ph[:, :G],
                        w1_sb[:, c * DH + h * P: c * DH + (h + 1) * P],
                        xt[:, c * G:(c + 1) * G],
                        start=(c == 0), stop=(c == NDC - 1))
                nc.scalar.activation(actT[:, h * G:(h + 1) * G], ph[:, :G],
                                     AF.Gelu, bias=b1_sb[:, h:h + 1], scale=1.0)

            # MM2: outT[d, t] over 4 d-chunks; LoRA-up opens each PSUM group
            for dc in range(NDC):
                po = ps_o.tile([P, G_MAX], f32, tag="ps_o", name=f"po{gi}_{dc}")
                nc.tensor.matmul(po[:, :G],
                                 wu_sb[:, dc * P:(dc + 1) * P],
                                 g2[:, :G],
                                 start=True, stop=False)
                # (wu rows >= ER are zero, so g2's padded rows are free)
                for h in range(NH):
                    nc.tensor.matmul(
                        po[:, :G],
                        w2_sb[:, h * D + dc * P: h * D + (dc + 1) * P],
                        actT[:, h * G:(h + 1) * G],
                        start=False, stop=(h == NH - 1))
                o_sb = outp.tile([P, G_MAX], bf16, tag="o_sb", name=f"o{gi}_{dc}")
                nc.vector.tensor_scalar(o_sb[:, :G], po[:, :G],
                                        b2_sb[:, dc:dc + 1], None, op0=OP.add)
                nc.sync.dma_start(outT_r[:, dc, gt0:gt0 + G], o_sb[:, :G])

    nc.compile()
    return nc


def _get_nc():
    key = ("v2", TC)
    if key not in _CACHE:
        _CACHE[key] = _build(TC)
    return _CACHE[key]


def _make_in_maps(inputs, tc_tokens=TC, n_cores=N_CORES):
    import ml_dtypes
    bf = ml_dtypes.bfloat16

    x = np.ascontiguousarray(inputs["x"], dtype=np.float32)
    T = x.size // D
    xT_full = np.ascontiguousarray(x.reshape(T, D).T.astype(bf))  # [D, T]
    W1 = np.ascontiguousarray(np.asarray(inputs["W1"], dtype=np.float32).astype(bf))
    W2 = np.ascontiguousarray(np.asarray(inputs["W2"], dtype=np.float32).astype(bf))
    b1 = np.ascontiguousarray(
        np.asarray(inputs["b1"], dtype=np.float32).reshape(NH, P).T)  # [P, NH]
    b2 = np.ascontiguousarray(
        np.asarray(inputs["b2"], dtype=np.float32).reshape(NDC, P).T)  # [P, NDC]
    wdn = np.zeros((D, P), dtype=bf)
    wdn[:, :ER] = (np.asarray(inputs["w_down"], dtype=np.float32)
                   .transpose(1, 0, 2).reshape(D, ER).astype(bf))
    wup = np.zeros((P, D), dtype=bf)
    wup[:ER] = np.asarray(inputs["w_up"], dtype=np.float32).reshape(ER, D).astype(bf)

    # routing weights: wts[e, t] = sum_k topk_probs[t, k] * (topk_indices[t, k] == e)
    tkp = np.asarray(inputs["topk_probs"], dtype=np.float32)  # [T, K]
    tki = np.asarray(inputs["topk_indices"])  # [T, K]
    wts = np.zeros((E, T), dtype=np.float32)
    for e in range(E):
        wts[e] = np.sum(np.where(tki == e, tkp, 0.0), axis=1)
    wtse_full = np.zeros((32, T), dtype=np.float32)
    wtse_full[:ER] = np.repeat(wts, R, axis=0)  # [ER, T], zero-padded to 32 rows

    in_maps = []
    for c in range(n_cores):
        sl = slice(c * tc_tokens, (c + 1) * tc_tokens)
        in_maps.append(dict(
            xT=np.ascontiguousarray(xT_full[:, sl]),
            w1=W1, w2=W2, b1=b1, b2=b2, wd=wdn, wu=wup,
            wtse=np.ascontiguousarray(wtse_full[:, sl])))
    return in_maps


def _ensure_ntff_hook():
    """Register the axon NTFF profile hook if the image's antenv lacks it."""
    import sys
    import types
    try:
        from antenv.axon_hooks import get_axon_ntff_profile_hook  # noqa: F401
        return True
    except ImportError:
        pass
    try:
        from trn_agent_boot.trn_boot import _ntff_profile_via_ctypes
        mod = types.ModuleType("antenv.axon_hooks")
        _hook = [None]
        mod.set_axon_ntff_profile_hook = lambda h: _hook.__setitem__(0, h)
        mod.get_axon_ntff_profile_hook = lambda: _hook[0]
        sys.modules["antenv.axon_hooks"] = mod
        import antenv
        antenv.axon_hooks = mod
        mod.set_axon_ntff_profile_hook(
            _ntff_profile_via_ctypes("/opt/axon/libaxon_pjrt.so"))
        return True
    except Exception:
        return False


def kernel(**inputs):
    from concourse.bass_utils import run_bass_kernel_spmd

    nc = _get_nc()
    in_maps = _make_in_maps(inputs)
    trace = bool(int(os.environ.get("KERNEL_TRACE", "0")))
    if trace and not _ensure_ntff_hook():
        trace = False
    res = run_bass_kernel_spmd(nc, in_maps, list(range(N_CORES)), trace=trace)
    if trace:
        _CACHE["last_result"] = res
    outT = np.concatenate(
        [np.asarray(res.results[i]["outT"]) for i in range(N_CORES)], axis=1)
    out = outT.astype(np.float32).T  # [T, D]
    return np.ascontiguousarray(out).reshape(np.asarray(inputs["x"]).shape)


# revision 21
# speedup vs baseline: 1.0582x; 1.0111x over previous
"""Trainium2 Bass kernel: ConvNeXt MLP + parallel MoE-LoRA (data-parallel over tokens).

Math per token t (D=512, Dh=2048, E=3 experts, r=8, top-k=2):
    base = gelu(x @ W1 + b1) @ W2 + b2
    g_e  = gelu(x @ w_down[e]) * wts[e, t]          (wts from top-k routing)
    out  = base + sum_e g_e @ w_up[e]

v2 strategy (per NeuronCore, 8 cores data-parallel on the token dim):
  Fully feature-major ("transposed") dataflow so the PE does nothing but
  512-free-dim bf16 matmuls back to back:
    - host ships x^T [D, Tc] pre-cast to bf16 -> no on-chip transposes/casts.
    - MM1: hT[h, t] = W1_chunk.T @ xT, ScalarE fuses +b1 and gelu, writing
      actT in bf16 (feature-major).
    - MM2 computes the TRANSPOSED output: outT[d, t] = W2_chunk.T @ actT
      accumulated over 16 h-chunks, plus the rank-24 LoRA 'up' matmul in the
      same PSUM group.  b2 becomes a per-partition scalar -> fused into the
      DVE drain (PSUM -> bf16 SBUF -> DMA out as outT [D, Tc]).
    - LoRA: gT[24, t] = gelu(wdown_all.T @ xT) on 24 partitions, scaled by
      routing weights with one DVE tensor_tensor against a host-prepared
      wtsE [24, Tc] tile.
    - routing weights (mask-combine of topk_probs/indices) and all weight
      layout/casts are host-side prep; output is transposed back on host.
  PE work per 512-token group: 4 (lora down) + 64 (MM1) + 4x17 (MM2) = 136
  bf16 matmuls, free dim 512, no other PE instructions.
"""

import os
import numpy as np

P = 128
D = 512
DH = 2048
E = 3
R = 8
ER = E * R  # 24
NH = DH // P  # 16
NDC = D // P  # 4
N_CORES = 8
T_FULL = 64 * 28 * 28  # 50176
TC = T_FULL // N_CORES  # 6272
GROUP_TILES = 4
G_MAX = GROUP_TILES * P  # 512

_CACHE = {}


def _build(tc_tokens):
    import concourse.bacc as bacc
    import concourse.tile as tile
    import concourse.mybir as mybir
    from contextlib import ExitStack

    f32 = mybir.dt.float32
    bf16 = mybir.dt.bfloat16
    AF = mybir.ActivationFunctionType
    OP = mybir.AluOpType

    nt = tc_tokens // P
    assert tc_tokens % P == 0

    nc = bacc.Bacc("TRN2", target_bir_lowering=False, debug=False,
                   num_devices=N_CORES)

    # group schedule: full 512-token groups + one remainder group
    groups = []
    t0 = 0
    while t0 < nt:
        ng = min(GROUP_TILES, nt - t0)
        groups.append((t0 * P, ng * P))
        t0 += ng
    NG = len(groups)

    # All big inputs are host-relaid so each DMA moves fat contiguous
    # per-partition lines (4-16 KiB descriptors):
    #   xg:  [NG*128, 4*512]  group-blocked x^T, [p, c*512+t]
    #   w1h: [128, 16*512]    [p, h*512 + c*128 + m] (h-major: MM1 can start
    #                         after the first h-quarter lands)
    #   w2h: [128, 16*512]    [p, h*512 + dc*128 + m]
    xg = nc.dram_tensor("xg", [NG * P, NDC * G_MAX], bf16, kind="ExternalInput").ap()
    w1h = nc.dram_tensor("w1h", [P, NH * D], bf16, kind="ExternalInput").ap()
    w2h = nc.dram_tensor("w2h", [P, NH * D], bf16, kind="ExternalInput").ap()
    b1 = nc.dram_tensor("b1", [P, NH], f32, kind="ExternalInput").ap()
    b2 = nc.dram_tensor("b2", [P, NDC], f32, kind="ExternalInput").ap()
    # LoRA weights zero-padded to full 128 partitions/columns on host so the
    # LoRA matmuls are uniform 128x128x512 (avoids LDWEIGHTS row/col-group
    # conflict stalls against the neighbouring full-size matmuls)
    wd = nc.dram_tensor("wd", [D, P], bf16, kind="ExternalInput").ap()
    wu = nc.dram_tensor("wu", [P, D], bf16, kind="ExternalInput").ap()
    wtse = nc.dram_tensor("wtse", [32, tc_tokens], f32, kind="ExternalInput").ap()
    outT = nc.dram_tensor("outT", [D, tc_tokens], bf16, kind="ExternalOutput").ap()

    outT_r = outT.rearrange("(c p) t -> p c t", p=P)

    with tile.TileContext(nc) as tc, ExitStack() as ctx:
        cons = ctx.enter_context(tc.tile_pool(name="cons", bufs=1))
        xin = ctx.enter_context(tc.tile_pool(name="xin", bufs=4))
        actp = ctx.enter_context(tc.tile_pool(name="actp", bufs=2))
        gp = ctx.enter_context(tc.tile_pool(name="gp", bufs=3))
        outp = ctx.enter_context(tc.tile_pool(name="outp", bufs=6))
        ps_h = ctx.enter_context(tc.tile_pool(name="ps_h", bufs=4, space="PSUM"))
        ps_o = ctx.enter_context(tc.tile_pool(name="ps_o", bufs=3, space="PSUM"))
        ps_g = ctx.enter_context(tc.tile_pool(name="ps_g", bufs=1, space="PSUM"))

        # ---- PE warm-up first: tiny matmuls on a memset tile during the
        # input DMA window (HAM reaches K=8/8 before the real matmuls start)
        warm = cons.tile([P, 16], bf16)
        nc.vector.memset(warm[:], 0.0)
        psw = ps_g.tile([16, 16], f32, tag="ps_g", name="warm")
        for i in range(40):
            nc.tensor.matmul(psw[:], warm[:, :16], warm[:, :16],
                             start=True, stop=True)

        # ---- weight / const DMAs (w1 h-quarters first) ----
        wd_sb = cons.tile([P, NDC * P], bf16)
        nc.sync.dma_start(wd_sb.rearrange("p (c r) -> p c r", c=NDC),
                          wd.rearrange("(c p) r -> p c r", p=P))
        b1_sb = cons.tile([P, NH], f32)
        nc.sync.dma_start(b1_sb[:], b1)
        b2_sb = cons.tile([P, NDC], f32)
        nc.sync.dma_start(b2_sb[:], b2)
        wu_sb = cons.tile([P, D], bf16)
        nc.sync.dma_start(wu_sb[:], wu)

        w1_sb = cons.tile([P, NH * D], bf16)
        HQ = NH * D // 4
        nc.sync.dma_start(w1_sb[:, 0:HQ], w1h[:, 0:HQ])

        x_tiles = {}

        def fetch_group(gi, nsplit=1):
            gt0, gG = groups[gi]
            xt = xin.tile([P, NDC * G_MAX], bf16, tag="xT", name=f"xT{gi}")
            w = NDC * G_MAX // nsplit
            for s in range(nsplit):
                nc.sync.dma_start(xt[:, s * w:(s + 1) * w],
                                  xg[gi * P:(gi + 1) * P, s * w:(s + 1) * w])
            x_tiles[gi] = xt

        fetch_group(0, nsplit=4)
        for hq in range(1, 4):
            nc.sync.dma_start(w1_sb[:, hq * HQ:(hq + 1) * HQ],
                              w1h[:, hq * HQ:(hq + 1) * HQ])
        fetch_group(1, nsplit=2)

        w2_sb = cons.tile([P, NH * D], bf16)
        for hq in range(4):
            nc.sync.dma_start(w2_sb[:, hq * HQ:(hq + 1) * HQ],
                              w2h[:, hq * HQ:(hq + 1) * HQ])

        wtse_sb = cons.tile([32, tc_tokens], f32)
        nc.sync.dma_start(wtse_sb[:], wtse)

        if len(groups) > 2:
            fetch_group(2)

        # ---- main loop over groups ----
        for gi, (gt0, G) in enumerate(groups):
            if gi + 3 < len(groups) and (gi + 3) not in x_tiles:
                fetch_group(gi + 3)
            xt = x_tiles.pop(gi)

            # LoRA down first: its ScalarE/DVE chain runs under MM1
            pg = ps_g.tile([P, G_MAX], f32, tag="ps_g", name=f"pg{gi}")
            for c in range(NDC):
                nc.tensor.matmul(pg[:, :G],
                                 wd_sb[:, c * P:(c + 1) * P],
                                 xt[:, c * G_MAX:c * G_MAX + G],
                                 start=(c == 0), stop=(c == NDC - 1))
            g_f = gp.tile([32, G_MAX], f32, tag="g_f", name=f"g_f{gi}")
            nc.scalar.activation(g_f[:, :G], pg[:32, :G], AF.Gelu)
            g2 = gp.tile([P, G_MAX], bf16, tag="g2", name=f"g2{gi}")
            nc.vector.memset(g2[:, :G], 0.0)
            nc.vector.tensor_tensor(g2[:32, :G], g_f[:, :G],
                                    wtse_sb[:, gt0:gt0 + G], op=OP.mult)

            # MM1 + fused bias+gelu -> actT bf16 (feature-major)
            actT = actp.tile([P, NH * G], bf16, tag="actT", name=f"actT{gi}")
            for h in range(NH):
                ph = ps_h.tile([P, G_MAX], f32, tag="ps_h", name=f"ph{gi}_{h}")
                for c in range(NDC):
                    nc.tensor.matmul(
                        ph[:, :G],
                        w1_sb[:, h * D + c * P: h * D + (c + 1) * P],
                        xt[:, c * G_MAX:c * G_MAX + G],
                        start=(c == 0), stop=(c == NDC - 1))
                nc.scalar.activation(actT[:, h * G:(h + 1) * G], ph[:, :G],
                                     AF.Gelu, bias=b1_sb[:, h:h + 1], scale=1.0)

            # MM2: outT[d, t] over 4 d-chunks; LoRA-up opens each PSUM group
            for dc in range(NDC):
                po = ps_o.tile([P, G_MAX], f32, tag="ps_o", name=f"po{gi}_{dc}")
                nc.tensor.matmul(po[:, :G],
                                 wu_sb[:, dc * P:(dc + 1) * P],
                                 g2[:, :G],
                                 start=True, stop=False)
                # (wu rows >= ER are zero, so g2's padded rows are free)
                for h in range(NH):
                    nc.tensor.matmul(
                        po[:, :G],
                        w2_sb[:, h * D + dc * P: h * D + (dc + 1) * P],
                        actT[:, h * G:(h + 1) * G],
                        start=False, stop=(h == NH - 1))
                # (w2_sb layout [p, h*512 + dc*128 + m] matches this slice)
                o_sb = outp.tile([P, G_MAX], bf16, tag="o_sb", name=f"o{gi}_{dc}")
                nc.vector.tensor_scalar(o_sb[:, :G], po[:, :G],
                                        b2_sb[:, dc:dc + 1], None, op0=OP.add)
                nc.sync.dma_start(outT_r[:, dc, gt0:gt0 + G], o_sb[:, :G])

    nc.compile()
    return nc


def _get_nc():
    key = ("v2", TC)
    if key not in _CACHE:
        _CACHE[key] = _build(TC)
    return _CACHE[key]


def _make_in_maps(inputs, tc_tokens=TC, n_cores=N_CORES):
    import ml_dtypes
    bf = ml_dtypes.bfloat16

    x = np.ascontiguousarray(inputs["x"], dtype=np.float32)
    T = x.size // D
    xT_full = np.ascontiguousarray(x.reshape(T, D).T.astype(bf))  # [D, T]
    # w1h[p, h*512 + c*128 + m] = W1[c*128+p, h*128+m]
    W1h = np.ascontiguousarray(
        np.asarray(inputs["W1"], dtype=np.float32).astype(bf)
        .reshape(NDC, P, NH, P).transpose(1, 2, 0, 3).reshape(P, NH * D))
    # w2h[p, h*512 + dc*128 + m] = W2[h*128+p, dc*128+m]
    W2h = np.ascontiguousarray(
        np.asarray(inputs["W2"], dtype=np.float32).astype(bf)
        .reshape(NH, P, NDC, P).transpose(1, 0, 2, 3).reshape(P, NH * D))
    b1 = np.ascontiguousarray(
        np.asarray(inputs["b1"], dtype=np.float32).reshape(NH, P).T)  # [P, NH]
    b2 = np.ascontiguousarray(
        np.asarray(inputs["b2"], dtype=np.float32).reshape(NDC, P).T)  # [P, NDC]
    wdn = np.zeros((D, P), dtype=bf)
    wdn[:, :ER] = (np.asarray(inputs["w_down"], dtype=np.float32)
                   .transpose(1, 0, 2).reshape(D, ER).astype(bf))
    wup = np.zeros((P, D), dtype=bf)
    wup[:ER] = np.asarray(inputs["w_up"], dtype=np.float32).reshape(ER, D).astype(bf)

    # routing weights: wts[e, t] = sum_k topk_probs[t, k] * (topk_indices[t, k] == e)
    tkp = np.asarray(inputs["topk_probs"], dtype=np.float32)  # [T, K]
    tki = np.asarray(inputs["topk_indices"])  # [T, K]
    wts = np.zeros((E, T), dtype=np.float32)
    for e in range(E):
        wts[e] = np.sum(np.where(tki == e, tkp, 0.0), axis=1)
    wtse_full = np.zeros((32, T), dtype=np.float32)
    wtse_full[:ER] = np.repeat(wts, R, axis=0)  # [ER, T], zero-padded to 32 rows

    # group-blocked x^T: xg[g*128+p, c*512+t] = xT[c*128+p, g*512+t]
    GM = GROUP_TILES * P  # 512
    ngf = tc_tokens // GM  # full groups per core
    rem = tc_tokens - ngf * GM
    ng = ngf + (1 if rem else 0)

    in_maps = []
    for c in range(n_cores):
        sl = slice(c * tc_tokens, (c + 1) * tc_tokens)
        xT_c = xT_full[:, sl]  # [D, tc]
        xg = np.zeros((ng * P, NDC * GM), dtype=bf)
        xg[:ngf * P] = (xT_c[:, :ngf * GM].reshape(NDC, P, ngf, GM)
                        .transpose(2, 1, 0, 3).reshape(ngf * P, NDC * GM))
        if rem:
            tail = xT_c[:, ngf * GM:].reshape(NDC, P, rem)
            blk = xg[ngf * P:].reshape(P, NDC, GM)
            for cc in range(NDC):
                blk[:, cc, :rem] = tail[cc]
        in_maps.append(dict(
            xg=xg, w1h=W1h, w2h=W2h, b1=b1, b2=b2, wd=wdn, wu=wup,
            wtse=np.ascontiguousarray(wtse_full[:, sl])))
    return in_maps


def _ensure_ntff_hook():
    """Register the axon NTFF profile hook if the image's antenv lacks it."""
    import sys
    import types
    try:
        from antenv.axon_hooks import get_axon_ntff_profile_hook  # noqa: F401
        return True
    except ImportError:
        pass
    try:
        from trn_agent_boot.trn_boot import _ntff_profile_via_ctypes
        mod = types.ModuleType("antenv.axon_hooks")
        _hook = [None]
        mod.set_axon_ntff_profile_hook = lambda h: _hook.__setitem__(0, h)
        mod.get_axon_ntff_profile_hook = lambda: _hook[0]
        sys.modules["antenv.axon_hooks"] = mod
        import antenv
        antenv.axon_hooks = mod
        mod.set_axon_ntff_profile_hook(
            _ntff_profile_via_ctypes("/opt/axon/libaxon_pjrt.so"))
        return True
    except Exception:
        return False


def kernel(**inputs):
    from concourse.bass_utils import run_bass_kernel_spmd

    nc = _get_nc()
    in_maps = _make_in_maps(inputs)
    trace = bool(int(os.environ.get("KERNEL_TRACE", "0")))
    if trace and not _ensure_ntff_hook():
        trace = False
    res = run_bass_kernel_spmd(nc, in_maps, list(range(N_CORES)), trace=trace)
    if trace:
        _CACHE["last_result"] = res
    outT = np.concatenate(
        [np.asarray(res.results[i]["outT"]) for i in range(N_CORES)], axis=1)
    out = outT.astype(np.float32).T  # [T, D]
    return np.ascontiguousarray(out).reshape(np.asarray(inputs["x"]).shape)
